# revision 1
# baseline (speedup 1.0000x reference)
"""MoE (top-2 of 8 experts) Trainium2 kernel, expert-parallel across 8 NeuronCores.

Sharding: expert-parallel. Core e holds expert e's weights and receives the
tokens routed to it (host-side all-to-all by routing decision, per the
sharding hint). On device, each core computes the routing logits for its
tokens, the top-2 softmax gate for its own expert (selection is encoded in a
host-provided +-1 one-hot difference vector, so the device never does an
argmax), the gated expert matmul (fp32r on the PE at full rate), and its
expert's contribution to the importance sum for the load-balancing loss.

Host side does only sharding/unsharding: gather/scatter-add of token rows
(each token's output is the sum of its two expert contributions) and the
final 8-element cv^2 reduction for the loss scalar.
"""

import numpy as np

E = 8
D = 1024
H = 4096
LOSS_COEF = 0.01
EPS = 1e-10
NK = D // 128  # K chunks
NH = H // 512  # N chunks


def _build(t_pad: int, with_bias: bool):
    from concourse import bacc, mybir
    import concourse.tile as tile

    mch = t_pad // 128
    f32 = mybir.dt.float32
    f32r = mybir.dt.float32r
    AF = mybir.ActivationFunctionType

    nc = bacc.Bacc(None)
    xt_d = nc.declare_dram_parameter("xt", [D, t_pad], f32r, isOutput=False)
    w_d = nc.declare_dram_parameter("w", [D, H], f32r, isOutput=False)
    wg_d = nc.declare_dram_parameter("wg", [D, E], f32r, isOutput=False)
    sel_d = nc.declare_dram_parameter("sel", [t_pad, E], f32, isOutput=False)
    pm_d = nc.declare_dram_parameter("pm", [t_pad, 1], f32, isOutput=False)
    if with_bias:
        bb_d = nc.declare_dram_parameter("bb", [128, H], f32, isOutput=False)
    u_d = nc.declare_dram_parameter("u", [t_pad, H], f32, isOutput=True)
    imp_d = nc.declare_dram_parameter("imp", [1, 1], f32, isOutput=True)

    with tile.TileContext(nc) as tc:
        with (
            tc.tile_pool(name="xt", bufs=1) as xt_pool,
            tc.tile_pool(name="cst", bufs=1) as cst_pool,
            tc.tile_pool(name="gat", bufs=1) as gat_pool,
            tc.tile_pool(name="wts", bufs=3) as w_pool,
            tc.tile_pool(name="out", bufs=4) as out_pool,
            tc.tile_pool(name="psm", bufs=4, space="PSUM") as psm,
            tc.tile_pool(name="psl", bufs=2, space="PSUM") as psl,
            tc.tile_pool(name="psi", bufs=1, space="PSUM") as psi,
        ):
            # ---- static SBUF residents ----
            xt_sb = xt_pool.tile([128, NK, t_pad], f32r)
            for k in range(NK):
                nc.sync.dma_start(xt_sb[:, k, :], xt_d[k * 128 : (k + 1) * 128, :])
            wg_sb = cst_pool.tile([128, NK, E], f32r)
            for k in range(NK):
                nc.sync.dma_start(wg_sb[:, k, :], wg_d[k * 128 : (k + 1) * 128, :])
            sel_sb = cst_pool.tile([128, mch, E], f32)
            nc.sync.dma_start(
                sel_sb[:], sel_d.rearrange("(m p) e -> p m e", p=128)
            )
            pm_sb = cst_pool.tile([128, mch, 1], f32)
            nc.sync.dma_start(pm_sb[:], pm_d.rearrange("(m p) o -> p m o", p=128))
            ones_sb = cst_pool.tile([128, 1], f32)
            nc.vector.memset(ones_sb[:], 1.0)
            if with_bias:
                bb_sb = cst_pool.tile([128, H], f32)
                nc.sync.dma_start(bb_sb[:], bb_d[:])

            # ---- gates: logits -> sigmoid(l_own - l_other) * padmask ----
            gcol = gat_pool.tile([128, mch], f32)  # per-token gate, col per m-chunk
            imp_ps = psi.tile([1, 1], f32)
            for m in range(mch):
                lps = psl.tile([128, E], f32)
                for k in range(NK):
                    nc.tensor.matmul(
                        lps[:],
                        xt_sb[:, k, m * 128 : (m + 1) * 128],
                        wg_sb[:, k, :],
                        start=(k == 0),
                        stop=(k == NK - 1),
                    )
                scr = gat_pool.tile([128, E], f32, tag="scr")
                ld = gat_pool.tile([128, 1], f32, tag="ld")
                # scr = logits * sel ; ld = row-sum(scr) = l_own - l_other
                nc.vector.scalar_tensor_tensor(
                    scr[:],
                    lps[:],
                    1.0,
                    sel_sb[:, m, :],
                    op0=mybir.AluOpType.mult,
                    op1=mybir.AluOpType.mult,
                    accum_out=ld[:],
                )
                g = gat_pool.tile([128, 1], f32, tag="g")
                nc.scalar.activation(g[:], ld[:], AF.Sigmoid)
                nc.vector.tensor_mul(gcol[:, m : m + 1], g[:], pm_sb[:, m, :])
                # importance += sum over tokens of gate
                nc.tensor.matmul(
                    imp_ps[:],
                    gcol[:, m : m + 1],
                    ones_sb[:],
                    start=(m == 0),
                    stop=(m == mch - 1),
                )
            imp_sb = gat_pool.tile([1, 1], f32, tag="impsb")
            nc.vector.tensor_copy(imp_sb[:], imp_ps[:])
            nc.sync.dma_start(imp_d[:], imp_sb[:])

            # ---- main expert matmul: u = diag(g) (X @ W) (+ g*b) ----
            for n in range(NH):
                wt = w_pool.tile([128, NK, 512], f32r)
                for k in range(NK):
                    nc.sync.dma_start(
                        wt[:, k, :],
                        w_d[k * 128 : (k + 1) * 128, n * 512 : (n + 1) * 512],
                    )
                for m in range(mch):
                    ps = psm.tile([128, 512], f32)
                    for k in range(NK):
                        nc.tensor.matmul(
                            ps[:],
                            xt_sb[:, k, m * 128 : (m + 1) * 128],
                            wt[:, k, :],
                            start=(k == 0),
                            stop=(k == NK - 1),
                        )
                    ot = out_pool.tile([128, 512], f32)
                    nc.scalar.activation(
                        ot[:], ps[:], AF.Copy, scale=gcol[:, m : m + 1]
                    )
                    if with_bias:
                        # ot += g * b  (u row = g*(xW) + g*b)
                        nc.vector.scalar_tensor_tensor(
                            ot[:],
                            bb_sb[:, n * 512 : (n + 1) * 512],
                            gcol[:, m : m + 1],
                            ot[:],
                            op0=mybir.AluOpType.mult,
                            op1=mybir.AluOpType.add,
                        )
                    nc.sync.dma_start(
                        u_d[m * 128 : (m + 1) * 128, n * 512 : (n + 1) * 512],
                        ot[:],
                    )
    nc.compile()
    return nc


def _cv_squared(v: np.ndarray) -> np.float32:
    v = v.astype(np.float32)
    return np.float32(v.var(ddof=1) / (v.mean() ** 2 + EPS))


def kernel(x, w_gate, expert_w, expert_b):
    from concourse.bass_utils import run_bass_kernel_spmd

    x = np.asarray(x, dtype=np.float32)
    w_gate = np.asarray(w_gate, dtype=np.float32)
    expert_w = np.ascontiguousarray(np.asarray(expert_w, dtype=np.float32))
    expert_b = np.asarray(expert_b, dtype=np.float32)
    B, S, _ = x.shape
    N = B * S
    xf = x.reshape(N, D)

    # ---- host routing (sharding decision only; fp64 so the top-2 selection
    # matches the fp32 reference even through near-ties) ----
    logits = xf.astype(np.float64) @ w_gate.astype(np.float64)
    order = np.argsort(-logits, axis=1, kind="stable")
    top1, top2 = order[:, 0].copy(), order[:, 1].copy()

    idx = [np.nonzero((top1 == e) | (top2 == e))[0] for e in range(E)]
    counts = np.array([len(i) for i in idx], dtype=np.int64)
    t_pad = max(128, int(-(-counts.max() // 128) * 128))

    with_bias = bool(np.any(expert_b))
    nc = _build(t_pad, with_bias)

    in_maps = []
    for e in range(E):
        ids = idx[e]
        t = len(ids)
        xt = np.zeros((D, t_pad), np.float32)
        xt[:, :t] = xf[ids].T
        sel = np.zeros((t_pad, E), np.float32)
        rows = np.arange(t)
        sel[rows, e] = 1.0
        other = np.where(top1[ids] == e, top2[ids], top1[ids])
        sel[rows, other] -= 1.0
        pm = np.zeros((t_pad, 1), np.float32)
        pm[:t] = 1.0
        m = {
            "xt": xt,
            "w": expert_w[e],
            "wg": w_gate,
            "sel": sel,
            "pm": pm,
        }
        if with_bias:
            m["bb"] = np.broadcast_to(expert_b[e], (128, H)).copy()
        in_maps.append(m)

    res = run_bass_kernel_spmd(nc, in_maps, list(range(E)))
    kernel.last_results = res

    # ---- unshard: scatter-add the two expert contributions per token ----
    y = np.zeros((N, H), np.float32)
    imp = np.zeros(E, np.float32)
    for e in range(E):
        u = res.results[e]["u"]
        y[idx[e]] += u[: counts[e]]
        imp[e] = res.results[e]["imp"][0, 0]
    load = counts.astype(np.float32)
    loss = np.float32(LOSS_COEF) * (_cv_squared(imp) + _cv_squared(load))
    return y.reshape(B, S, H), np.float32(loss)
